# revision 7
# baseline (speedup 1.0000x reference)
"""MeshGCN on 8 Trainium2 NeuronCores (Bass/Tile).

Math shortcut: the reference's hidden loop overwrites `out` and always convolves
the same `x`, so only Wh[4]/bh[4] matter:
    h1 = relu((Dis @ A_hat @ Dis @ x) @ W4 + b4)        A_hat = A + I (by dst)
    y  = (Dis @ A_hat @ Dis @ (h1 @ W_out)) + b_out
with Dis = diag(1/sqrt(indeg+1)). dis[src] is folded into the gathered table and
dis[dst] applied per node after aggregation; the self-loop becomes the node's
own (pre-scaled) row. Each conv is then gather + segment-sum + scale + tiny GEMM.

Distribution: dst-shard nodes over 8 cores (62500 each, plus dummy padding to
490 groups of 128). Nodes are degree-sorted per core so each group of 128 nodes
shares a padded degree D; gathers are indirect DMAs of 128 table rows per
instruction (one per padded-degree column), reduced on DVE with a strided view.
A packed PE pipeline (transpose -> block-diag W4 -> relu -> block-diag W_out ->
transpose) handles 5 groups per pass. Launch 1 emits each core's packed h2s
table (1MB); the host concatenates all 8 and launch 2 aggregates it into y.
"""
import sys
sys.path.insert(0, "/opt/trn_rl_repo")

import numpy as np

import concourse.bass as bass
import concourse.bacc as bacc
import concourse.mybir as mybir
import concourse.tile as tile
from concourse.bass_utils import run_bass_kernel_spmd

F32 = mybir.dt.float32
I32 = mybir.dt.int32

N = 500_000
H = 24
HP = 4            # padded out channels (OUT=3)
NC = 8            # cores
CN = N // NC      # real nodes per core = 62500
PB = 5            # groups per PE pack
NG = 490          # groups per core (62720 slots >= 62500)
SLOTS = NG * 128
NPACK = NG // PB  # 98
ROWS = NPACK * 128  # packed h2s rows per core (12544)
PW = PB * HP      # packed row width (20)
ZROW = N          # zeros row index in the xs table

_R = np.array([0, 0, 0, 1, 1, 2])
_C = np.array([0, 1, 2, 1, 2, 2])


def _run(nc, maps):
    try:
        return run_bass_kernel_spmd(nc, maps, list(range(NC)), trace=True)
    except Exception:
        return run_bass_kernel_spmd(nc, maps, list(range(NC)), trace=False)


# ---------------------------------------------------------------- builders

def _build_nc1(Ds, G1):
    """Launch 1: MP1 + feature transform -> packed h2s [ROWS, PW] per core."""
    nc = bacc.Bacc()
    xs = nc.declare_dram_parameter("xs", [N + 1, H], F32, isOutput=False)
    xself = nc.declare_dram_parameter("xself", [SLOTS, H], F32, isOutput=False)
    idx1 = nc.declare_dram_parameter("idx1", [128, max(G1, 1)], I32, isOutput=False)
    disg = nc.declare_dram_parameter("disg", [128, NG], F32, isOutput=False)
    dis4 = nc.declare_dram_parameter("dis4", [NPACK, 128, PW], F32, isOutput=False)
    w4b = nc.declare_dram_parameter("w4b", [PB * H, PB * H], F32, isOutput=False)
    wob = nc.declare_dram_parameter("wob", [PB * H, PW], F32, isOutput=False)
    b4p = nc.declare_dram_parameter("b4p", [PB * H, 1], F32, isOutput=False)
    iden = nc.declare_dram_parameter("iden", [128, 128], F32, isOutput=False)
    h2s = nc.declare_dram_parameter("h2s", [128, NPACK * PW], F32, isOutput=True)

    with tile.TileContext(nc) as tc:
        with (
            tc.tile_pool(name="stat", bufs=1) as stat,
            tc.tile_pool(name="idxp", bufs=3) as idxp,
            tc.tile_pool(name="gat", bufs=6) as gat,
            tc.tile_pool(name="work", bufs=3) as work,
            tc.tile_pool(name="psum", bufs=2, space="PSUM") as psum,
        ):
            ident = stat.tile([128, 128], F32)
            nc.sync.dma_start(out=ident[:], in_=iden[:, :])
            w4t = stat.tile([PB * H, PB * H], F32)
            nc.sync.dma_start(out=w4t[:], in_=w4b[:, :])
            wot = stat.tile([PB * H, PW], F32)
            nc.sync.dma_start(out=wot[:], in_=wob[:, :])
            b4t = stat.tile([PB * H, 1], F32)
            nc.sync.dma_start(out=b4t[:], in_=b4p[:, :])
            dist = stat.tile([128, NG], F32)
            nc.sync.dma_start(out=dist[:], in_=disg[:, :])
            stash = stat.tile([128, NPACK * PW], F32)
            idxall = stat.tile([128, max(G1, 1)], I32)
            nc.sync.dma_start(out=idxall[:], in_=idx1[:, :])

            col = 0
            for t in range(NPACK):
                pack = work.tile([128, PB * H], F32, tag="pack")
                for b in range(PB):
                    g = t * PB + b
                    D = Ds[g]
                    st = work.tile([128, H], F32, tag="self")
                    nc.sync.dma_start(out=st[:], in_=xself[g * 128:(g + 1) * 128, :])
                    if D > 0:
                        gt = gat.tile([128, D * H], F32, tag="gt")
                        for k in range(D):
                            nc.gpsimd.indirect_dma_start(
                                out=gt[:, k * H:(k + 1) * H],
                                out_offset=None,
                                in_=xs[:, :],
                                in_offset=bass.IndirectOffsetOnAxis(
                                    ap=idxall[:, col + k:col + k + 1], axis=0),
                            )
                        red = work.tile([128, H], F32, tag="red")
                        if D > 1:
                            gv = gt[:].rearrange("p (k c) -> p c k", k=D)
                            nc.vector.reduce_sum(out=red[:], in_=gv, axis=mybir.AxisListType.X)
                            nc.vector.tensor_add(out=red[:], in0=red[:], in1=st[:])
                        else:
                            nc.vector.tensor_add(out=red[:], in0=gt[:, :H], in1=st[:])
                        src_sum = red[:]
                    else:
                        src_sum = st[:]
                    nc.vector.tensor_scalar_mul(
                        out=pack[:, b * H:(b + 1) * H],
                        in0=src_sum,
                        scalar1=dist[:, g:g + 1],
                    )
                    col += D

                aggT_ps = psum.tile([PB * H, 128], F32, tag="aggT")
                nc.tensor.transpose(out=aggT_ps[:], in_=pack[:], identity=ident[:])
                aggT = work.tile([PB * H, 128], F32, tag="aggT_sb")
                nc.scalar.copy(out=aggT[:], in_=aggT_ps[:])
                h1_ps = psum.tile([PB * H, 128], F32, tag="h1")
                nc.tensor.matmul(out=h1_ps[:], lhsT=w4t[:], rhs=aggT[:], start=True, stop=True)
                h1T = work.tile([PB * H, 128], F32, tag="h1_sb")
                nc.scalar.activation(
                    out=h1T[:], in_=h1_ps[:],
                    func=mybir.ActivationFunctionType.Relu,
                    bias=b4t[:], scale=1.0,
                )
                h2_ps = psum.tile([PW, 128], F32, tag="h2")
                nc.tensor.matmul(out=h2_ps[:], lhsT=wot[:], rhs=h1T[:], start=True, stop=True)
                h2T = work.tile([PW, 128], F32, tag="h2_sb")
                nc.scalar.copy(out=h2T[:], in_=h2_ps[:])
                h2n_ps = psum.tile([128, PW], F32, tag="h2n")
                nc.tensor.transpose(out=h2n_ps[:], in_=h2T[:], identity=ident[:PW, :PW])
                d4 = work.tile([128, PW], F32, tag="d4")
                nc.sync.dma_start(out=d4[:], in_=dis4[t])
                nc.vector.tensor_mul(
                    out=stash[:, t * PW:(t + 1) * PW], in0=h2n_ps[:], in1=d4[:],
                )

            nc.sync.dma_start(out=h2s[:, :], in_=stash[:])
    nc.compile()
    return nc


def _build_nc2(Ds, G2):
    """Launch 2: MP2 over the full packed h2s table -> packed y [ROWS, PW]."""
    TROWS = NC * 128 + 1  # + zeros row
    nc = bacc.Bacc()
    tbl = nc.declare_dram_parameter("tbl", [TROWS, NPACK * PW], F32, isOutput=False)
    idx2 = nc.declare_dram_parameter("idx2", [128, max(G2, 1)], I32, isOutput=False)
    disg = nc.declare_dram_parameter("disg", [128, NG], F32, isOutput=False)
    boutp = nc.declare_dram_parameter("boutp", [128, HP], F32, isOutput=False)
    selfh = nc.declare_dram_parameter("selfh", [128, NPACK * PW], F32, isOutput=False)
    yout = nc.declare_dram_parameter("yout", [128, NPACK * PW], F32, isOutput=True)

    with tile.TileContext(nc) as tc:
        with (
            tc.tile_pool(name="stat", bufs=1) as stat,
            tc.tile_pool(name="idxp", bufs=3) as idxp,
            tc.tile_pool(name="gat", bufs=6) as gat,
            tc.tile_pool(name="work", bufs=3) as work,
        ):
            dist = stat.tile([128, NG], F32)
            nc.sync.dma_start(out=dist[:], in_=disg[:, :])
            bt = stat.tile([128, HP], F32)
            nc.sync.dma_start(out=bt[:], in_=boutp[:, :])
            selft = stat.tile([128, NPACK * PW], F32)
            nc.sync.dma_start(out=selft[:], in_=selfh[:, :])
            ystash = stat.tile([128, NPACK * PW], F32)
            idxall = stat.tile([128, max(G2, 1)], I32)
            nc.sync.dma_start(out=idxall[:], in_=idx2[:, :])

            flat = tbl[:, :].rearrange("r c -> (r c)")[:, None]

            col = 0
            for t in range(NPACK):
                for b in range(PB):
                    g = t * PB + b
                    D = Ds[g]
                    sslice = selft[:, (t * PW + b * HP):(t * PW + (b + 1) * HP)]
                    if D > 0:
                        gt = gat.tile([128, D * HP], F32, tag="gt")
                        for k in range(D):
                            nc.gpsimd.indirect_dma_start(
                                out=gt[:, k * HP:(k + 1) * HP],
                                out_offset=None,
                                in_=flat,
                                in_offset=bass.IndirectOffsetOnAxis(
                                    ap=idxall[:, col + k:col + k + 1], axis=0),
                            )
                        red = work.tile([128, HP], F32, tag="red")
                        if D > 1:
                            gv = gt[:].rearrange("p (k c) -> p c k", k=D)
                            nc.vector.reduce_sum(out=red[:], in_=gv, axis=mybir.AxisListType.X)
                            rsum = red[:]
                        else:
                            rsum = gt[:, :HP]
                        acc = work.tile([128, HP], F32, tag="acc")
                        nc.vector.tensor_add(out=acc[:], in0=rsum, in1=sslice)
                        base = acc[:]
                    else:
                        base = sslice
                    ys = work.tile([128, HP], F32, tag="ys")
                    nc.vector.tensor_scalar_mul(
                        out=ys[:], in0=base, scalar1=dist[:, g:g + 1],
                    )
                    nc.vector.tensor_add(
                        out=ystash[:, (t * PW + b * HP):(t * PW + (b + 1) * HP)],
                        in0=ys[:], in1=bt[:],
                    )
                    col += D

            nc.sync.dma_start(out=yout[:, :], in_=ystash[:])
    nc.compile()
    return nc


# ---------------------------------------------------------------- host side

def _prep(featr3, stmdist, edge_index):
    f0 = featr3[:, 0][:, _R, _C]
    f1 = featr3[:, 1][:, _R, _C]
    f2 = featr3[:, 2].reshape(-1, 9)
    x = np.concatenate([f0, f1, f2, stmdist], axis=1).astype(np.float32)

    src = np.asarray(edge_index[0], dtype=np.int64)
    dst = np.asarray(edge_index[1], dtype=np.int64)
    indeg = np.bincount(dst, minlength=N).astype(np.int64)
    dis = (1.0 / np.sqrt(indeg + 1.0)).astype(np.float32)
    xs = np.empty((N + 1, H), dtype=np.float32)
    xs[:N] = dis[:, None] * x
    xs[N] = 0.0

    core = dst // CN
    local = dst % CN

    slotmaps = np.empty((NC, CN), dtype=np.int64)   # core, local -> slot
    nodeat = np.full((NC, SLOTS), -1, dtype=np.int64)  # core, slot -> local
    for c in range(NC):
        dloc = indeg[c * CN:(c + 1) * CN]
        order = np.argsort(dloc, kind="stable")
        slotmaps[c][order] = np.arange(SLOTS - CN, SLOTS)
        nodeat[c][slotmaps[c]] = np.arange(CN)

    eslot = slotmaps[core, local]                   # [E]
    Dsc = np.zeros((NC, NG), dtype=np.int64)
    for c in range(NC):
        cnt = np.bincount(eslot[core == c], minlength=SLOTS)
        Dsc[c] = cnt.reshape(NG, 128).max(axis=1)
    Ds = Dsc.max(axis=0)
    colbase = np.concatenate([[0], np.cumsum(Ds)]).astype(np.int64)
    G1 = int(colbase[-1])
    G2 = G1

    # global node id -> flat f32 position of its h2s row in the packed table
    sl_glob = slotmaps[src // CN, src % CN]
    t_g = sl_glob // (PB * 128)
    b_g = (sl_glob // 128) % PB
    p_g = sl_glob % 128
    rw = NPACK * PW
    srcflat = ((src // CN) * 128 + p_g) * rw + t_g * PW + b_g * HP
    zflat = (NC * 128) * rw

    in1, in2 = [], []
    for c in range(NC):
        m = np.flatnonzero(core == c)
        es, esrc, esf = eslot[m], src[m], srcflat[m]
        o = np.argsort(es, kind="stable")
        es, esrc, esf = es[o], esrc[o], esf[o]
        starts = np.searchsorted(es, np.arange(SLOTS))
        rank = np.arange(len(es)) - starts[es]
        g = es // 128
        p = es % 128

        idx1 = np.full((128, max(G1, 1)), ZROW, dtype=np.int32)
        idx1[p, colbase[g] + rank] = esrc.astype(np.int32)

        idx2 = np.full((128, max(G2, 1)), zflat, dtype=np.int32)
        idx2[p, colbase[g] + rank] = esf.astype(np.int32)
        own_local = nodeat[c]
        valid = own_local >= 0

        disv = np.zeros(SLOTS, dtype=np.float32)
        disv[valid] = dis[c * CN + own_local[valid]]
        dgrid = disv.reshape(NG, 128)
        disg_t = np.ascontiguousarray(dgrid.T)      # [128, NG]

        dis4 = np.zeros((NPACK, 128, PW), dtype=np.float32)
        for b in range(PB):
            dis4[:, :, b * HP:(b + 1) * HP] = dgrid[b::PB][:NPACK][:, :, None]

        xself = np.zeros((SLOTS, H), dtype=np.float32)
        xself[valid] = xs[c * CN + own_local[valid]]

        in1.append({"xs": xs, "xself": xself, "idx1": idx1, "disg": disg_t,
                    "dis4": dis4})
        in2.append({"idx2": idx2, "disg": disg_t})

    return in1, in2, Ds, G1, G2, nodeat


def kernel(featr3, stmdist, edge_index, Wh, bh, W_out, b_out):
    in1, in2, Ds, G1, G2, nodeat = _prep(
        np.asarray(featr3), np.asarray(stmdist), np.asarray(edge_index))

    W4 = np.asarray(Wh)[4].astype(np.float32)
    b4 = np.asarray(bh)[4].astype(np.float32)
    Wo = np.zeros((H, HP), dtype=np.float32)
    Wo[:, :3] = np.asarray(W_out).astype(np.float32)
    bo = np.zeros(HP, dtype=np.float32)
    bo[:3] = np.asarray(b_out).astype(np.float32)

    w4b = np.kron(np.eye(PB, dtype=np.float32), W4).astype(np.float32)
    wob = np.kron(np.eye(PB, dtype=np.float32), Wo).astype(np.float32)
    b4p = np.tile(b4, PB)[:, None].astype(np.float32)
    boutp = np.tile(bo[None, :], (128, 1)).astype(np.float32)

    Ds_l = [int(d) for d in Ds]

    nc1 = _build_nc1(Ds_l, G1)
    iden = np.eye(128, dtype=np.float32)
    maps1 = [dict(in1[c], w4b=w4b, wob=wob, b4p=b4p, iden=iden) for c in range(NC)]
    r1 = _run(nc1, maps1)
    h2s_all = np.concatenate([r1.results[c]["h2s"] for c in range(NC)], axis=0)
    tbl = np.concatenate(
        [h2s_all, np.zeros((1, NPACK * PW), np.float32)], axis=0)

    nc2 = _build_nc2(Ds_l, G2)
    maps2 = [dict(in2[c], tbl=tbl, boutp=boutp,
                  selfh=np.ascontiguousarray(tbl[c * 128:(c + 1) * 128]))
             for c in range(NC)]
    r2 = _run(nc2, maps2)

    y = np.empty((N, 3), dtype=np.float32)
    for c in range(NC):
        yp = r2.results[c]["yout"].reshape(128, NPACK, PB, HP)
        ys = yp.transpose(1, 2, 0, 3).reshape(SLOTS, HP)  # slot-major
        own_local = nodeat[c]
        valid = own_local >= 0
        y[c * CN + own_local[valid]] = ys[valid][:, :3]

    kernel.exec_time_ns = (getattr(r1, "exec_time_ns", 0) or 0) + \
        (getattr(r2, "exec_time_ns", 0) or 0)
    return y


# revision 8
# speedup vs baseline: 1.0042x; 1.0042x over previous
"""MeshGCN on 8 Trainium2 NeuronCores (Bass/Tile).

Math shortcut: the reference's hidden loop overwrites `out` and always convolves
the same `x`, so only Wh[4]/bh[4] matter:
    h1 = relu((Dis @ A_hat @ Dis @ x) @ W4 + b4)        A_hat = A + I (by dst)
    y  = (Dis @ A_hat @ Dis @ (h1 @ W_out)) + b_out
with Dis = diag(1/sqrt(indeg+1)). dis[src] is folded into the gathered table and
dis[dst] applied per node after aggregation; the self-loop becomes the node's
own (pre-scaled) row. Each conv is then gather + segment-sum + scale + tiny GEMM.

Distribution: dst-shard nodes over 8 cores (62500 each, plus dummy padding to
490 groups of 128). Nodes are degree-sorted per core so each group of 128 nodes
shares a padded degree D; gathers are indirect DMAs of 128 table rows per
instruction (one per padded-degree column), reduced on DVE with a strided view.
A packed PE pipeline (transpose -> block-diag W4 -> relu -> block-diag W_out ->
transpose) handles 5 groups per pass. Launch 1 emits each core's packed h2s
table (1MB); the host concatenates all 8 and launch 2 aggregates it into y.
"""
import sys
sys.path.insert(0, "/opt/trn_rl_repo")

import numpy as np

import concourse.bass as bass
import concourse.bacc as bacc
import concourse.mybir as mybir
import concourse.tile as tile
from concourse.bass_utils import run_bass_kernel_spmd

F32 = mybir.dt.float32
I32 = mybir.dt.int32

N = 500_000
H = 24
HP = 4            # padded out channels (OUT=3)
NC = 8            # cores
CN = N // NC      # real nodes per core = 62500
PB = 5            # groups per PE pack
NG = 490          # groups per core (62720 slots >= 62500)
SLOTS = NG * 128
NPACK = NG // PB  # 98
ROWS = NPACK * 128  # packed h2s rows per core (12544)
PW = PB * HP      # packed row width (20)
ZROW = N          # zeros row index in the xs table

_R = np.array([0, 0, 0, 1, 1, 2])
_C = np.array([0, 1, 2, 1, 2, 2])


def _run(nc, maps):
    try:
        return run_bass_kernel_spmd(nc, maps, list(range(NC)), trace=True)
    except Exception:
        return run_bass_kernel_spmd(nc, maps, list(range(NC)), trace=False)


# ---------------------------------------------------------------- builders

def _build_nc1(Ds, G1):
    """Launch 1: MP1 + feature transform -> packed h2s [ROWS, PW] per core."""
    nc = bacc.Bacc()
    xs = nc.declare_dram_parameter("xs", [N + 1, H], F32, isOutput=False)
    xself = nc.declare_dram_parameter("xself", [SLOTS, H], F32, isOutput=False)
    idx1 = nc.declare_dram_parameter("idx1", [128, max(G1, 1)], I32, isOutput=False)
    disg = nc.declare_dram_parameter("disg", [128, NG], F32, isOutput=False)
    dis4 = nc.declare_dram_parameter("dis4", [NPACK, 128, PW], F32, isOutput=False)
    w4b = nc.declare_dram_parameter("w4b", [PB * H, PB * H], F32, isOutput=False)
    wob = nc.declare_dram_parameter("wob", [PB * H, PW], F32, isOutput=False)
    b4p = nc.declare_dram_parameter("b4p", [PB * H, 1], F32, isOutput=False)
    iden = nc.declare_dram_parameter("iden", [128, 128], F32, isOutput=False)
    h2s = nc.declare_dram_parameter("h2s", [128, NPACK * PW], F32, isOutput=True)

    with tile.TileContext(nc) as tc:
        with (
            tc.tile_pool(name="stat", bufs=1) as stat,
            tc.tile_pool(name="idxp", bufs=3) as idxp,
            tc.tile_pool(name="gat", bufs=6) as gat,
            tc.tile_pool(name="work", bufs=3) as work,
            tc.tile_pool(name="psum", bufs=2, space="PSUM") as psum,
        ):
            ident = stat.tile([128, 128], F32)
            nc.sync.dma_start(out=ident[:], in_=iden[:, :])
            w4t = stat.tile([PB * H, PB * H], F32)
            nc.sync.dma_start(out=w4t[:], in_=w4b[:, :])
            wot = stat.tile([PB * H, PW], F32)
            nc.sync.dma_start(out=wot[:], in_=wob[:, :])
            b4t = stat.tile([PB * H, 1], F32)
            nc.sync.dma_start(out=b4t[:], in_=b4p[:, :])
            dist = stat.tile([128, NG], F32)
            nc.sync.dma_start(out=dist[:], in_=disg[:, :])
            stash = stat.tile([128, NPACK * PW], F32)
            idxall = stat.tile([128, max(G1, 1)], I32)
            nc.sync.dma_start(out=idxall[:], in_=idx1[:, :])

            col = 0
            for t in range(NPACK):
                pack = work.tile([128, PB * H], F32, tag="pack")
                for b in range(PB):
                    g = t * PB + b
                    D = Ds[g]
                    st = work.tile([128, H], F32, tag="self")
                    nc.sync.dma_start(out=st[:], in_=xself[g * 128:(g + 1) * 128, :])
                    if D > 0:
                        gt = gat.tile([128, D * H], F32, tag="gt")
                        for k in range(D):
                            nc.gpsimd.indirect_dma_start(
                                out=gt[:, k * H:(k + 1) * H],
                                out_offset=None,
                                in_=xs[:, :],
                                in_offset=bass.IndirectOffsetOnAxis(
                                    ap=idxall[:, col + k:col + k + 1], axis=0),
                            )
                        red = work.tile([128, H], F32, tag="red")
                        if D > 1:
                            gv = gt[:].rearrange("p (k c) -> p c k", k=D)
                            nc.vector.reduce_sum(out=red[:], in_=gv, axis=mybir.AxisListType.X)
                            nc.vector.tensor_add(out=red[:], in0=red[:], in1=st[:])
                        else:
                            nc.vector.tensor_add(out=red[:], in0=gt[:, :H], in1=st[:])
                        src_sum = red[:]
                    else:
                        src_sum = st[:]
                    nc.vector.tensor_scalar_mul(
                        out=pack[:, b * H:(b + 1) * H],
                        in0=src_sum,
                        scalar1=dist[:, g:g + 1],
                    )
                    col += D

                aggT_ps = psum.tile([PB * H, 128], F32, tag="aggT")
                nc.tensor.transpose(out=aggT_ps[:], in_=pack[:], identity=ident[:])
                aggT = work.tile([PB * H, 128], F32, tag="aggT_sb")
                nc.scalar.copy(out=aggT[:], in_=aggT_ps[:])
                h1_ps = psum.tile([PB * H, 128], F32, tag="h1")
                nc.tensor.matmul(out=h1_ps[:], lhsT=w4t[:], rhs=aggT[:], start=True, stop=True)
                h1T = work.tile([PB * H, 128], F32, tag="h1_sb")
                nc.scalar.activation(
                    out=h1T[:], in_=h1_ps[:],
                    func=mybir.ActivationFunctionType.Relu,
                    bias=b4t[:], scale=1.0,
                )
                h2_ps = psum.tile([PW, 128], F32, tag="h2")
                nc.tensor.matmul(out=h2_ps[:], lhsT=wot[:], rhs=h1T[:], start=True, stop=True)
                h2T = work.tile([PW, 128], F32, tag="h2_sb")
                nc.scalar.copy(out=h2T[:], in_=h2_ps[:])
                h2n_ps = psum.tile([128, PW], F32, tag="h2n")
                nc.tensor.transpose(out=h2n_ps[:], in_=h2T[:], identity=ident[:PW, :PW])
                d4 = work.tile([128, PW], F32, tag="d4")
                nc.sync.dma_start(out=d4[:], in_=dis4[t])
                nc.vector.tensor_mul(
                    out=stash[:, t * PW:(t + 1) * PW], in0=h2n_ps[:], in1=d4[:],
                )

            nc.sync.dma_start(out=h2s[:, :], in_=stash[:])
    nc.compile()
    return nc


def _build_nc2(Ds, G2):
    """Launch 2: MP2 over the full packed h2s table -> packed y [ROWS, PW]."""
    TROWS = NC * 128 + 1  # + zeros row
    nc = bacc.Bacc()
    tbl = nc.declare_dram_parameter("tbl", [TROWS, NPACK * PW], F32, isOutput=False)
    idx2 = nc.declare_dram_parameter("idx2", [128, max(G2, 1)], I32, isOutput=False)
    disg = nc.declare_dram_parameter("disg", [128, NG], F32, isOutput=False)
    boutp = nc.declare_dram_parameter("boutp", [128, HP], F32, isOutput=False)
    selfh = nc.declare_dram_parameter("selfh", [128, NPACK * PW], F32, isOutput=False)
    yout = nc.declare_dram_parameter("yout", [128, NPACK * PW], F32, isOutput=True)

    with tile.TileContext(nc) as tc:
        with (
            tc.tile_pool(name="stat", bufs=1) as stat,
            tc.tile_pool(name="idxp", bufs=3) as idxp,
            tc.tile_pool(name="gat", bufs=6) as gat,
            tc.tile_pool(name="work", bufs=3) as work,
        ):
            dist = stat.tile([128, NG], F32)
            nc.sync.dma_start(out=dist[:], in_=disg[:, :])
            bt = stat.tile([128, HP], F32)
            nc.sync.dma_start(out=bt[:], in_=boutp[:, :])
            selft = stat.tile([128, NPACK * PW], F32)
            nc.sync.dma_start(out=selft[:], in_=selfh[:, :])
            ystash = stat.tile([128, NPACK * PW], F32)
            idxall = stat.tile([128, max(G2, 1)], I32)
            nc.sync.dma_start(out=idxall[:], in_=idx2[:, :])

            flat = tbl[:, :].rearrange("r c -> (r c)")[:, None]

            col = 0
            for t in range(NPACK):
                for b in range(PB):
                    g = t * PB + b
                    D = Ds[g]
                    sslice = selft[:, (t * PW + b * HP):(t * PW + (b + 1) * HP)]
                    if D > 0:
                        gt = gat.tile([128, D * HP], F32, tag="gt")
                        for k in range(D):
                            nc.gpsimd.indirect_dma_start(
                                out=gt[:, k * HP:(k + 1) * HP],
                                out_offset=None,
                                in_=flat,
                                in_offset=bass.IndirectOffsetOnAxis(
                                    ap=idxall[:, col + k:col + k + 1], axis=0),
                            )
                        red = work.tile([128, HP], F32, tag="red")
                        if D > 1:
                            gv = gt[:].rearrange("p (k c) -> p c k", k=D)
                            nc.vector.reduce_sum(out=red[:], in_=gv, axis=mybir.AxisListType.X)
                            rsum = red[:]
                        else:
                            rsum = gt[:, :HP]
                        acc = work.tile([128, HP], F32, tag="acc")
                        nc.vector.tensor_add(out=acc[:], in0=rsum, in1=sslice)
                        base = acc[:]
                    else:
                        base = sslice
                    ys = work.tile([128, HP], F32, tag="ys")
                    nc.vector.tensor_scalar_mul(
                        out=ys[:], in0=base, scalar1=dist[:, g:g + 1],
                    )
                    nc.vector.tensor_add(
                        out=ystash[:, (t * PW + b * HP):(t * PW + (b + 1) * HP)],
                        in0=ys[:], in1=bt[:],
                    )
                    col += D

            nc.sync.dma_start(out=yout[:, :], in_=ystash[:])
    nc.compile()
    return nc


# ---------------------------------------------------------------- host side

def _prep(featr3, stmdist, edge_index):
    f0 = featr3[:, 0][:, _R, _C]
    f1 = featr3[:, 1][:, _R, _C]
    f2 = featr3[:, 2].reshape(-1, 9)
    x = np.concatenate([f0, f1, f2, stmdist], axis=1).astype(np.float32)

    src = np.asarray(edge_index[0], dtype=np.int64)
    dst = np.asarray(edge_index[1], dtype=np.int64)
    indeg = np.bincount(dst, minlength=N).astype(np.int64)
    dis = (1.0 / np.sqrt(indeg + 1.0)).astype(np.float32)
    xs = np.empty((N + 1, H), dtype=np.float32)
    xs[:N] = dis[:, None] * x
    xs[N] = 0.0

    # global degree-sorted round-robin: rank r -> core r % NC, so every core
    # sees an identical degree profile and the common padded schedule is tight
    S = np.argsort(indeg, kind="stable")
    pos = np.empty(N, dtype=np.int64)
    pos[S] = np.arange(N)
    corev = pos % NC
    slotv = (SLOTS - CN) + pos // NC          # dummies occupy slots [0, SLOTS-CN)

    nodeat = np.full((NC, SLOTS), -1, dtype=np.int64)  # core, slot -> global node
    q = np.arange(CN)
    for c in range(NC):
        nodeat[c, SLOTS - CN:] = S[q * NC + c]

    eslot = slotv[dst]
    ecore = corev[dst]
    Dsc = np.zeros((NC, NG), dtype=np.int64)
    for c in range(NC):
        cnt = np.bincount(eslot[ecore == c], minlength=SLOTS)
        Dsc[c] = cnt.reshape(NG, 128).max(axis=1)
    Ds = Dsc.max(axis=0)
    colbase = np.concatenate([[0], np.cumsum(Ds)]).astype(np.int64)
    G1 = int(colbase[-1])
    G2 = G1

    # global node id -> flat f32 position of its h2s row in the packed table
    rw = NPACK * PW
    t_a = slotv // (PB * 128)
    b_a = (slotv // 128) % PB
    p_a = slotv % 128
    flatv = (corev * 128 + p_a) * rw + t_a * PW + b_a * HP
    zflat = (NC * 128) * rw

    in1, in2 = [], []
    for c in range(NC):
        m = np.flatnonzero(ecore == c)
        es, esrc = eslot[m], src[m]
        o = np.argsort(es, kind="stable")
        es, esrc = es[o], esrc[o]
        starts = np.searchsorted(es, np.arange(SLOTS))
        rank = np.arange(len(es)) - starts[es]
        g = es // 128
        p = es % 128

        idx1 = np.full((128, max(G1, 1)), ZROW, dtype=np.int32)
        idx1[p, colbase[g] + rank] = esrc.astype(np.int32)
        idx2 = np.full((128, max(G2, 1)), zflat, dtype=np.int32)
        idx2[p, colbase[g] + rank] = flatv[esrc].astype(np.int32)

        own = nodeat[c]
        valid = own >= 0
        disv = np.zeros(SLOTS, dtype=np.float32)
        disv[valid] = dis[own[valid]]
        dgrid = disv.reshape(NG, 128)
        disg_t = np.ascontiguousarray(dgrid.T)

        dis4 = np.zeros((NPACK, 128, PW), dtype=np.float32)
        for b in range(PB):
            dis4[:, :, b * HP:(b + 1) * HP] = dgrid[b::PB][:NPACK][:, :, None]

        xself = np.zeros((SLOTS, H), dtype=np.float32)
        xself[valid] = xs[own[valid]]

        in1.append({"xs": xs, "xself": xself, "idx1": idx1, "disg": disg_t,
                    "dis4": dis4})
        in2.append({"idx2": idx2, "disg": disg_t})

    return in1, in2, Ds, G1, G2, nodeat


def kernel(featr3, stmdist, edge_index, Wh, bh, W_out, b_out):
    in1, in2, Ds, G1, G2, nodeat = _prep(
        np.asarray(featr3), np.asarray(stmdist), np.asarray(edge_index))

    W4 = np.asarray(Wh)[4].astype(np.float32)
    b4 = np.asarray(bh)[4].astype(np.float32)
    Wo = np.zeros((H, HP), dtype=np.float32)
    Wo[:, :3] = np.asarray(W_out).astype(np.float32)
    bo = np.zeros(HP, dtype=np.float32)
    bo[:3] = np.asarray(b_out).astype(np.float32)

    w4b = np.kron(np.eye(PB, dtype=np.float32), W4).astype(np.float32)
    wob = np.kron(np.eye(PB, dtype=np.float32), Wo).astype(np.float32)
    b4p = np.tile(b4, PB)[:, None].astype(np.float32)
    boutp = np.tile(bo[None, :], (128, 1)).astype(np.float32)

    Ds_l = [int(d) for d in Ds]

    nc1 = _build_nc1(Ds_l, G1)
    iden = np.eye(128, dtype=np.float32)
    maps1 = [dict(in1[c], w4b=w4b, wob=wob, b4p=b4p, iden=iden) for c in range(NC)]
    r1 = _run(nc1, maps1)
    h2s_all = np.concatenate([r1.results[c]["h2s"] for c in range(NC)], axis=0)
    tbl = np.concatenate(
        [h2s_all, np.zeros((1, NPACK * PW), np.float32)], axis=0)

    nc2 = _build_nc2(Ds_l, G2)
    maps2 = [dict(in2[c], tbl=tbl, boutp=boutp,
                  selfh=np.ascontiguousarray(tbl[c * 128:(c + 1) * 128]))
             for c in range(NC)]
    r2 = _run(nc2, maps2)

    y = np.empty((N, 3), dtype=np.float32)
    for c in range(NC):
        yp = r2.results[c]["yout"].reshape(128, NPACK, PB, HP)
        ys = yp.transpose(1, 2, 0, 3).reshape(SLOTS, HP)  # slot-major
        own = nodeat[c]
        valid = own >= 0
        y[own[valid]] = ys[valid][:, :3]

    kernel.exec_time_ns = (getattr(r1, "exec_time_ns", 0) or 0) + \
        (getattr(r2, "exec_time_ns", 0) or 0)
    return y
